# revision 15
# baseline (speedup 1.0000x reference)
"""Distributed Trainium2 kernel for nn_ContrastiveLoss (survival contrastive loss).

Strategy (8 NeuronCores, symmetric fp8):
  host: quantile-bin rows into 4 risk groups (2048 each), stable-sort by
        group, L2-normalize, scale by 16 and cast to fp8e4 (e4m3); ship a
        rolled copy to each core so its supertile-rows sit at fixed virtual
        positions (SPMD-static program).
  device (core c): sim is symmetric, so only supertile pairs (I, I+d) for
        virtual I in {0,8}, d = 0..8 / 0..7 are computed — over 8 rolled
        copies this covers all 136 unordered 512x512 supertile pairs once.
        fp8 DoubleRow matmuls (K=256/matmul) -> psum; ACT exp (scale 10/256)
        in 3-tile batches with f32 accum row-sums; fp8 exp tiles feed
        DoubleRow ones-matmul column-sums (the mirror contribution) and a
        DVE reduce of tiles d=1..3 (group-boundary corrections).
  host: assemble per-row pos/den sums from row-accums, boundary reduces and
        colsums; subtract the exact diagonal exp(10*||z8||^2/256) computed
        from the shipped fp8 values; loss = mean(log den - log pos).
"""
import sys

sys.path.insert(0, "/opt/trn_rl_repo")
import numpy as np
import ml_dtypes

N, D, G, NCORES = 8192, 512, 4, 8
CT = 512               # supertile width
NT = N // CT           # 16 supertiles
SCALE = 16.0           # fp8 pre-scale
ESC = 10.0 / (SCALE * SCALE)   # exp scale applied to psum
F8NP = ml_dtypes.float8_e4m3

_built = None


def _build():
    from concourse import bacc, tile, mybir

    nc = bacc.Bacc(None, target_bir_lowering=False)
    f32 = mybir.dt.float32
    f8 = mybir.dt.float8e4
    AF = mybir.ActivationFunctionType
    AX = mybir.AxisListType
    PM = mybir.MatmulPerfMode.DoubleRow

    et = nc.dram_tensor("et", [128, 4, N], f8, kind="ExternalInput")
    ones2 = nc.dram_tensor("ones2", [128, 2, 16], f8, kind="ExternalInput")
    rsums = nc.dram_tensor("rsums", [128, 70], f32, kind="ExternalOutput")
    csum = nc.dram_tensor("csum", [15, 512], f32, kind="ExternalOutput")

    with tile.TileContext(nc) as tc:
        with tc.tile_pool(name="z", bufs=1) as zp, \
             tc.tile_pool(name="cst", bufs=1) as cst, \
             tc.tile_pool(name="eb", bufs=2) as ebp, \
             tc.tile_pool(name="pm", bufs=2, space="PSUM") as pmp, \
             tc.tile_pool(name="pc", bufs=2, space="PSUM") as pcp:

            nc.scalar.add_instruction(
                mybir.InstLoadActFuncSet(
                    name=nc.get_next_instruction_name(),
                    act_func_set_id=6, ins=[], outs=[]))

            o2 = cst.tile([128, 2, 16], f8)
            z8 = zp.tile([128, 4, N], f8)
            # progressive column chunks so compute starts after ~3.6us;
            # ones2 (needed only for colsums ~10us in) goes mid-stream
            bounds = [0, CT, 3 * CT, 6 * CT, 9 * CT, 12 * CT, 16 * CT]
            for i, (lo, hi) in enumerate(zip(bounds, bounds[1:])):
                nc.sync.dma_start(z8[:, :, lo:hi], et[:, :, lo:hi])
                if i == 1:
                    nc.sync.dma_start(o2[:], ones2[:])
            rsums_t = cst.tile([128, 70], f32)
            cstage = cst.tile([1, 15 * CT], f32)

            s = 0
            for Ii, I in enumerate((0, 8)):
                maxd = 9 if I == 0 else 8
                batches = [(0, 1, 2), (3, 4, 5),
                           (6, 7, 8) if I == 0 else (6, 7)]
                # exp tiles: [rb, d, col] fp8
                expt = ebp.tile([128, 4, 9, CT], f8, tag="expt")
                # batch-major: only the first pass over rb waits on new
                # input columns, later batches reuse columns already loaded
                for b, ds in enumerate(batches):
                    for rb in range(4):
                        rs = slice(I * CT + rb * 128, I * CT + (rb + 1) * 128)
                        pm = pmp.tile([128, 3 * CT], f32, tag="pm")
                        acol = Ii * 12 + rb * 3 + b
                        first = I == 0 and rb <= 1 and b == 0
                        for di, d in enumerate(ds):
                            cs = slice((I + d) * CT, (I + d + 1) * CT)
                            for kk in range(2):
                                nc.tensor.matmul(
                                    pm[:, di * CT:(di + 1) * CT],
                                    z8[:, 2 * kk:2 * kk + 2, rs],
                                    z8[:, 2 * kk:2 * kk + 2, cs],
                                    start=(kk == 0), stop=(kk == 1),
                                    perf_mode=PM)
                            if first and di == 0:
                                # split: the first exps need only d0's columns
                                nc.scalar.activation(
                                    expt[:, rb, 0:1, :], pm[:, :CT],
                                    AF.Exp, scale=ESC,
                                    accum_out=rsums_t[:, acol:acol + 1])
                        if first:
                            nc.scalar.activation(
                                expt[:, rb, 1:3, :], pm[:, CT:3 * CT],
                                AF.Exp, scale=ESC,
                                accum_out=rsums_t[:, 48 + rb:49 + rb])
                        elif Ii == 1 and b == 1:
                            nc.scalar.activation(
                                expt[:, rb, ds[0]:ds[0] + len(ds), :],
                                pm[:, :len(ds) * CT], AF.Exp, scale=ESC)
                        else:
                            nc.scalar.activation(
                                expt[:, rb, ds[0]:ds[0] + len(ds), :],
                                pm[:, :len(ds) * CT], AF.Exp, scale=ESC,
                                accum_out=rsums_t[:, acol:acol + 1])
                    if b == 1:
                        # d=1..3 fp8 rowsums (group-boundary info); for I=8
                        # widen to d=1..5 so it also yields batch1's row-sum
                        for rb in range(4):
                            if Ii == 0:
                                rcol = 24 + rb * 3
                                nc.vector.tensor_reduce(
                                    rsums_t[:, rcol:rcol + 3],
                                    expt[:, rb, 1:4, :],
                                    axis=AX.X, op=mybir.AluOpType.add)
                            else:
                                rcol = 50 + rb * 5
                                nc.vector.tensor_reduce(
                                    rsums_t[:, rcol:rcol + 5],
                                    expt[:, rb, 1:6, :],
                                    axis=AX.X, op=mybir.AluOpType.add)
                    # colsums for completed off-diag tiles of this batch
                    for d in ds:
                        if d == 0:
                            continue
                        pc = pcp.tile([1, CT], f32, tag="pc")
                        for h in range(2):
                            nc.tensor.matmul(
                                pc[:], o2[:, :, 0:1],
                                expt[:, 2 * h:2 * h + 2, d, :],
                                start=(h == 0), stop=(h == 1), perf_mode=PM)
                        if s == 13:
                            nc.scalar.copy(
                                cstage[:, s * CT:(s + 1) * CT], pc[:])
                        else:
                            nc.vector.tensor_copy(
                                cstage[:, s * CT:(s + 1) * CT], pc[:])
                        s += 1
                        if s == 13:
                            nc.sync.dma_start(csum[:13, :],
                                              cstage[:, :13 * CT])
            assert s == 15
            nc.sync.dma_start(rsums[:], rsums_t[:])
            nc.sync.dma_start(csum[13:, :], cstage[:, 13 * CT:])

    nc.finalize()
    return nc


def _get_built():
    global _built
    if _built is None:
        _built = _build()
    return _built


def _host_prep(embeddings, survival_times):
    E = np.ascontiguousarray(np.asarray(embeddings, dtype=np.float32))
    t = np.asarray(survival_times, dtype=np.float32)
    q = np.quantile(t.astype(np.float64), [0.25, 0.5, 0.75])
    rg = (t[:, None].astype(np.float64) >= q[None, :]).sum(axis=1)
    counts = np.bincount(rg, minlength=G)
    assert (counts == N // G).all(), counts
    perm = np.argsort(rg, kind="stable")
    Es = E[perm]
    nrm = np.sqrt((Es.astype(np.float64) ** 2).sum(axis=1, keepdims=True))
    z = Es / np.maximum(nrm, 1e-12)
    z16 = (z * SCALE).astype(F8NP)          # [N, D] fp8
    zT = np.ascontiguousarray(z16.T)        # [D, N]
    ones2 = np.zeros((128, 2, 16), dtype=F8NP)
    ones2[:, :, 0] = 1.0
    in_maps = []
    for c in range(NCORES):
        roll = np.roll(zT, -c * CT, axis=1)               # [D, N]
        et = np.ascontiguousarray(
            roll.reshape(4, 128, N).transpose(1, 0, 2))    # [128, 4, N]
        in_maps.append({"et": et, "ones2": ones2})
    return in_maps, z16


def _host_combine(results, z16):
    tot = np.zeros(N, np.float64)
    pos = np.zeros(N, np.float64)
    for c in range(NCORES):
        rs_ = results[c]["rsums"].astype(np.float64)
        racc, rred = rs_[:, :24].copy(), rs_[:, 24:48].copy()
        racc[:, 0] += rs_[:, 48]     # (I=0, rb=0) batch0 was split in two
        racc[:, 3] += rs_[:, 49]     # (I=0, rb=1) batch0 was split in two
        for rb in range(4):
            R5 = rs_[:, 50 + rb * 5: 55 + rb * 5]
            rred[:, 12 + rb * 3: 15 + rb * 3] = R5[:, :3]
            racc[:, 13 + rb * 3] = R5[:, 2:5].sum(axis=1)
        csum = results[c]["csum"].astype(np.float64)
        s = 0
        for Ii, I in enumerate((0, 8)):
            aI = (I + c) % NT
            maxd = 9 if I == 0 else 8
            gI = aI // 4
            kp = 4 - (aI % 4)
            for rb in range(4):
                rows = slice(aI * CT + rb * 128, aI * CT + (rb + 1) * 128)
                A = racc[:, Ii * 12 + rb * 3: Ii * 12 + rb * 3 + 3]
                R = rred[:, Ii * 12 + rb * 3: Ii * 12 + rb * 3 + 3]
                tot[rows] += A.sum(axis=1)
                if kp == 1:
                    p = A[:, 0] - R[:, 0] - R[:, 1]
                elif kp == 2:
                    p = A[:, 0] - R[:, 1]
                elif kp == 3:
                    p = A[:, 0]
                else:
                    p = A[:, 0] + R[:, 2]
                pos[rows] += p
            for d in range(1, maxd):
                aJ = (I + d + c) % NT
                rows = slice(aJ * CT, (aJ + 1) * CT)
                tot[rows] += csum[s]
                if aJ // 4 == gI:
                    pos[rows] += csum[s]
                s += 1
    dlog = ESC * (z16.astype(np.float64) ** 2).sum(axis=1)
    dexp = np.exp(dlog)
    tot -= dexp
    pos -= dexp
    return np.float32(np.mean(np.log(tot) - np.log(pos)))


def kernel(embeddings, survival_times, censor):
    from concourse.bass_utils import run_bass_kernel_spmd

    nc = _get_built()
    in_maps, z16 = _host_prep(embeddings, survival_times)
    res = run_bass_kernel_spmd(nc, in_maps, list(range(NCORES)))
    return _host_combine(res.results, z16)


# revision 16
# speedup vs baseline: 1.0737x; 1.0737x over previous
"""Distributed Trainium2 kernel for nn_ContrastiveLoss (survival contrastive loss).

Strategy (8 NeuronCores, symmetric fp8):
  host: quantile-bin rows into 4 risk groups (2048 each), stable-sort by
        group, L2-normalize, scale by 16 and cast to fp8e4 (e4m3); ship a
        rolled copy to each core so its supertile-rows sit at fixed virtual
        positions (SPMD-static program).
  device (core c): sim is symmetric, so only supertile pairs (I, I+d) for
        virtual I in {0,8}, d = 0..8 / 0..7 are computed — over 8 rolled
        copies this covers all 136 unordered 512x512 supertile pairs once.
        fp8 DoubleRow matmuls (K=256/matmul) -> psum; ACT exp (scale 10/256)
        in 3-tile batches with f32 accum row-sums; fp8 exp tiles feed
        DoubleRow ones-matmul column-sums (the mirror contribution) and a
        DVE reduce of tiles d=1..3 (group-boundary corrections).
  host: assemble per-row pos/den sums from row-accums, boundary reduces and
        colsums; subtract the exact diagonal exp(10*||z8||^2/256) computed
        from the shipped fp8 values; loss = mean(log den - log pos).
"""
import sys

sys.path.insert(0, "/opt/trn_rl_repo")
import numpy as np
import ml_dtypes

N, D, G, NCORES = 8192, 512, 4, 8
CT = 512               # supertile width
NT = N // CT           # 16 supertiles
SCALE = 16.0           # fp8 pre-scale
ESC = 10.0 / (SCALE * SCALE)   # exp scale applied to psum
F8NP = ml_dtypes.float8_e4m3

_built = None


def _build():
    from concourse import bacc, tile, mybir

    nc = bacc.Bacc(None, target_bir_lowering=False)
    f32 = mybir.dt.float32
    f8 = mybir.dt.float8e4
    AF = mybir.ActivationFunctionType
    AX = mybir.AxisListType
    PM = mybir.MatmulPerfMode.DoubleRow

    et = nc.dram_tensor("et", [128, 4, N], f8, kind="ExternalInput")
    ones2 = nc.dram_tensor("ones2", [128, 2, 16], f8, kind="ExternalInput")
    rsums = nc.dram_tensor("rsums", [128, 50], f32, kind="ExternalOutput")
    csum = nc.dram_tensor("csum", [15, 512], f32, kind="ExternalOutput")

    with tile.TileContext(nc) as tc:
        with tc.tile_pool(name="z", bufs=1) as zp, \
             tc.tile_pool(name="cst", bufs=1) as cst, \
             tc.tile_pool(name="eb", bufs=2) as ebp, \
             tc.tile_pool(name="pm", bufs=2, space="PSUM") as pmp, \
             tc.tile_pool(name="pc", bufs=2, space="PSUM") as pcp:

            nc.scalar.add_instruction(
                mybir.InstLoadActFuncSet(
                    name=nc.get_next_instruction_name(),
                    act_func_set_id=6, ins=[], outs=[]))

            o2 = cst.tile([128, 2, 16], f8)
            z8 = zp.tile([128, 4, N], f8)
            # progressive column chunks so compute starts after ~3.6us;
            # ones2 (needed only for colsums ~10us in) goes mid-stream
            bounds = [0, CT, 3 * CT, 6 * CT, 9 * CT, 12 * CT, 16 * CT]
            for i, (lo, hi) in enumerate(zip(bounds, bounds[1:])):
                nc.sync.dma_start(z8[:, :, lo:hi], et[:, :, lo:hi])
                if i == 1:
                    nc.sync.dma_start(o2[:], ones2[:])
            rsums_t = cst.tile([128, 50], f32)
            cstage = cst.tile([1, 15 * CT], f32)

            s = 0
            for Ii, I in enumerate((0, 8)):
                maxd = 9 if I == 0 else 8
                batches = [(0, 1, 2), (3, 4, 5),
                           (6, 7, 8) if I == 0 else (6, 7)]
                # exp tiles: [rb, d, col] fp8
                expt = ebp.tile([128, 4, 9, CT], f8, tag="expt")
                # batch-major: only the first pass over rb waits on new
                # input columns, later batches reuse columns already loaded
                for b, ds in enumerate(batches):
                    for rb in range(4):
                        rs = slice(I * CT + rb * 128, I * CT + (rb + 1) * 128)
                        pm = pmp.tile([128, 3 * CT], f32, tag="pm")
                        acol = Ii * 12 + rb * 3 + b
                        first = I == 0 and rb <= 1 and b == 0
                        for di, d in enumerate(ds):
                            cs = slice((I + d) * CT, (I + d + 1) * CT)
                            for kk in range(2):
                                nc.tensor.matmul(
                                    pm[:, di * CT:(di + 1) * CT],
                                    z8[:, 2 * kk:2 * kk + 2, rs],
                                    z8[:, 2 * kk:2 * kk + 2, cs],
                                    start=(kk == 0), stop=(kk == 1),
                                    perf_mode=PM)
                            if first and di == 0:
                                # split: the first exps need only d0's columns
                                nc.scalar.activation(
                                    expt[:, rb, 0:1, :], pm[:, :CT],
                                    AF.Exp, scale=ESC,
                                    accum_out=rsums_t[:, acol:acol + 1])
                        if first:
                            nc.scalar.activation(
                                expt[:, rb, 1:3, :], pm[:, CT:3 * CT],
                                AF.Exp, scale=ESC,
                                accum_out=rsums_t[:, 48 + rb:49 + rb])
                        else:
                            nc.scalar.activation(
                                expt[:, rb, ds[0]:ds[0] + len(ds), :],
                                pm[:, :len(ds) * CT], AF.Exp, scale=ESC,
                                accum_out=rsums_t[:, acol:acol + 1])
                    if b == 1:
                        # d=1..3 fp8 rowsums (group-boundary info)
                        for rb in range(4):
                            rcol = Ii * 12 + rb * 3
                            nc.vector.tensor_reduce(
                                rsums_t[:, 24 + rcol:24 + rcol + 3],
                                expt[:, rb, 1:4, :],
                                axis=AX.X, op=mybir.AluOpType.add)
                    # colsums for completed off-diag tiles of this batch
                    for d in ds:
                        if d == 0:
                            continue
                        pc = pcp.tile([1, CT], f32, tag="pc")
                        for h in range(2):
                            nc.tensor.matmul(
                                pc[:], o2[:, :, 0:1],
                                expt[:, 2 * h:2 * h + 2, d, :],
                                start=(h == 0), stop=(h == 1), perf_mode=PM)
                        if s == 13:
                            nc.scalar.copy(
                                cstage[:, s * CT:(s + 1) * CT], pc[:])
                        else:
                            nc.vector.tensor_copy(
                                cstage[:, s * CT:(s + 1) * CT], pc[:])
                        s += 1
                        if s == 13:
                            nc.sync.dma_start(csum[:13, :],
                                              cstage[:, :13 * CT])
            assert s == 15
            nc.sync.dma_start(rsums[:], rsums_t[:])
            nc.sync.dma_start(csum[13:, :], cstage[:, 13 * CT:])

    nc.finalize()
    return nc


def _get_built():
    global _built
    if _built is None:
        _built = _build()
    return _built


def _host_prep(embeddings, survival_times):
    E = np.ascontiguousarray(np.asarray(embeddings, dtype=np.float32))
    t = np.asarray(survival_times, dtype=np.float32)
    q = np.quantile(t.astype(np.float64), [0.25, 0.5, 0.75])
    rg = (t[:, None].astype(np.float64) >= q[None, :]).sum(axis=1)
    counts = np.bincount(rg, minlength=G)
    assert (counts == N // G).all(), counts
    perm = np.argsort(rg, kind="stable")
    Es = E[perm]
    nrm = np.sqrt((Es.astype(np.float64) ** 2).sum(axis=1, keepdims=True))
    z = Es / np.maximum(nrm, 1e-12)
    z16 = (z * SCALE).astype(F8NP)          # [N, D] fp8
    zT = np.ascontiguousarray(z16.T)        # [D, N]
    ones2 = np.zeros((128, 2, 16), dtype=F8NP)
    ones2[:, :, 0] = 1.0
    in_maps = []
    for c in range(NCORES):
        roll = np.roll(zT, -c * CT, axis=1)               # [D, N]
        et = np.ascontiguousarray(
            roll.reshape(4, 128, N).transpose(1, 0, 2))    # [128, 4, N]
        in_maps.append({"et": et, "ones2": ones2})
    return in_maps, z16


def _host_combine(results, z16):
    tot = np.zeros(N, np.float64)
    pos = np.zeros(N, np.float64)
    for c in range(NCORES):
        rs_ = results[c]["rsums"].astype(np.float64)
        racc, rred = rs_[:, :24].copy(), rs_[:, 24:48]
        racc[:, 0] += rs_[:, 48]     # (I=0, rb=0) batch0 was split in two
        racc[:, 3] += rs_[:, 49]     # (I=0, rb=1) batch0 was split in two
        csum = results[c]["csum"].astype(np.float64)
        s = 0
        for Ii, I in enumerate((0, 8)):
            aI = (I + c) % NT
            maxd = 9 if I == 0 else 8
            gI = aI // 4
            kp = 4 - (aI % 4)
            for rb in range(4):
                rows = slice(aI * CT + rb * 128, aI * CT + (rb + 1) * 128)
                A = racc[:, Ii * 12 + rb * 3: Ii * 12 + rb * 3 + 3]
                R = rred[:, Ii * 12 + rb * 3: Ii * 12 + rb * 3 + 3]
                tot[rows] += A.sum(axis=1)
                if kp == 1:
                    p = A[:, 0] - R[:, 0] - R[:, 1]
                elif kp == 2:
                    p = A[:, 0] - R[:, 1]
                elif kp == 3:
                    p = A[:, 0]
                else:
                    p = A[:, 0] + R[:, 2]
                pos[rows] += p
            for d in range(1, maxd):
                aJ = (I + d + c) % NT
                rows = slice(aJ * CT, (aJ + 1) * CT)
                tot[rows] += csum[s]
                if aJ // 4 == gI:
                    pos[rows] += csum[s]
                s += 1
    dlog = ESC * (z16.astype(np.float64) ** 2).sum(axis=1)
    dexp = np.exp(dlog)
    tot -= dexp
    pos -= dexp
    return np.float32(np.mean(np.log(tot) - np.log(pos)))


def kernel(embeddings, survival_times, censor):
    from concourse.bass_utils import run_bass_kernel_spmd

    nc = _get_built()
    in_maps, z16 = _host_prep(embeddings, survival_times)
    res = run_bass_kernel_spmd(nc, in_maps, list(range(NCORES)))
    return _host_combine(res.results, z16)
